# revision 21
# baseline (speedup 1.0000x reference)
"""GAT node-level layer on 8 TRN2 NeuronCores.

Strategy: destination-sharded edge processing, one fused row-gather per edge.
 - Host (index-only preprocessing): sort edges by dst, shard by dst range
   (6250 nodes per core), window = 128 consecutive dst. Per-window chunk
   count cap_w = max over cores (uniform SPMD program). Edge slot (p, c):
   p = partition, c = chunk; per-slot arrays si (src id) and sg (dst rel,
   -1 for padding).
 - Phase 1 (per core shard): z_aug rows, fp16 layout
     [z(0:128) | es+ f32 bits(128:130) | es- f32 bits(130:132) | 1(132) | 0]
   where z = h @ W.T, s = z @ a_src, es+ = exp(s), es- = exp(0.01 s);
   lam = exp(-0.99 * q), q = z @ a_dst (lam stays local -- only needed for
   the core's own dst windows). All rows accumulate in SBUF (no per-tile
   DMAs; PE -> DVE / Scalar only), then two bulk DMAs build z_bounce and
   one AllGather produces z_full [N, 134] fp16.
 - Phase 2 per window w: gather 134-elem rows by src (one indirect DMA per
   128-edge chunk -- the SWDGE per-instruction launch cost is the wall);
   per-edge weight uses the identity
     exp(leaky_relu(s+q)) * exp(-q) = max(exp(s), exp(0.01 s) * exp(-0.99 q))
   (the exp(-q) factor cancels in the softmax), so no per-edge q data is
   needed: sel[p,d] = (sg==d) * max(es+[p], es-[p] * lam[d]), with the es
   f32 scalars read straight out of the gathered row via bitcast.
   PE accumulates po[d, 0:134] = sum_c sel_c.T @ g_c; col 132 (the gathered
   constant 1.0) gives the softmax denominator; normalize and write out.
Each chunk gathers into its own rotating tile so SWDGE launches never wait
on the previous chunk's transfer. No per-edge s/q gathers; the only
inter-core traffic is the z_aug AllGather.
"""

import sys

if "/opt/trn_rl_repo" not in sys.path:
    sys.path.insert(0, "/opt/trn_rl_repo")

from contextlib import ExitStack

import numpy as np

from concourse import bacc, bass, mybir, tile
from concourse.masks import make_identity

N_NODES = 50000
N_EDGES = 800000
D_IN = 256
D_OUT = 128
CORES = 8
P = 128
ROWW = 134  # gathered row, fp16 elems (268 B)
ESW = 6  # es-block width: es+ (2), es- (2), one, zero

F32 = mybir.dt.float32
F16 = mybir.dt.float16
I32 = mybir.dt.int32

_PROGRAM_CACHE: dict = {}


# ---------------------------------------------------------------- host prep
def preprocess_indices(src, dst, n_nodes=N_NODES, cores=CORES):
    """Sort edges by dst, shard by dst range, build per-core slot arrays
    si [128, TOTCH] (src node id) and sg [128, TOTCH] (dst&127, -1 pad),
    where window w owns chunks [starts[w], starts[w]+cap_w). Integer-only."""
    shard = n_nodes // cores
    wpc = (shard + P - 1) // P
    src = np.asarray(src).astype(np.int64)
    dst = np.asarray(dst).astype(np.int64)

    order = np.argsort(dst, kind="stable")
    ds = dst[order]
    # z_full row layout is per-core partition-major: node (core c, local r)
    # lives at row c*wpc*128 + (r%128)*wpc + r//128 (see phase-1 bulk write)
    s_raw = src[order]
    s_core = s_raw // shard
    s_r = s_raw % shard
    ss = s_core * (wpc * P) + (s_r & 127) * wpc + (s_r >> 7)
    bounds = np.searchsorted(ds, np.arange(cores + 1) * shard)

    percore = []
    counts_all = np.zeros((cores, wpc), np.int64)
    for c in range(cores):
        lo, hi = int(bounds[c]), int(bounds[c + 1])
        dloc = ds[lo:hi] - c * shard
        s_c = ss[lo:hi]
        w = dloc >> 7
        counts_all[c] = np.bincount(w, minlength=wpc)
        percore.append((dloc, s_c, w))

    cap_w = tuple(int(x) for x in (counts_all.max(axis=0) + P - 1) // P)
    starts = np.zeros(wpc, np.int64)
    starts[1:] = np.cumsum(cap_w)[:-1]
    totch = int(starts[-1] + cap_w[-1])

    arrs = []
    for c in range(cores):
        dloc, s_c, w = percore[c]
        wstarts = np.zeros(wpc, np.int64)
        wstarts[1:] = np.cumsum(counts_all[c])[:-1]
        pos_in_w = np.arange(len(dloc), dtype=np.int64) - wstarts[w]
        p = pos_in_w & 127
        ch = starts[w] + (pos_in_w >> 7)
        si = np.zeros((P, totch), np.int32)
        sg = np.full((P, totch), -1.0, np.float32)
        si[p, ch] = s_c
        sg[p, ch] = (dloc & 127).astype(np.float32)
        arrs.append({"si": si, "sg": sg})
    return cap_w, arrs


# ---------------------------------------------------------------- program
def build_program(cap_w, n_nodes=N_NODES, d_in=D_IN, d_out=D_OUT, cores=CORES):
    shard = n_nodes // cores
    wpc = (shard + P - 1) // P
    kc_n = d_in // P
    starts = np.zeros(wpc, np.int64)
    starts[1:] = np.cumsum(cap_w)[:-1]
    totch = int(starts[-1] + cap_w[-1])

    nc = bacc.Bacc(None, target_bir_lowering=False, debug=False)

    h_t = nc.dram_tensor("h_t", [d_in, shard], F32, kind="ExternalInput")
    w_d = nc.dram_tensor("W", [d_out, d_in], F32, kind="ExternalInput")
    a_d = nc.dram_tensor("a", [2 * d_out, 1], F32, kind="ExternalInput")
    si_d = nc.dram_tensor("si", [P, totch], I32, kind="ExternalInput")
    sg_d = nc.dram_tensor("sg", [P, totch], F32, kind="ExternalInput")
    out_d = nc.dram_tensor("out", [shard, d_out], F32, kind="ExternalOutput")

    rg = [list(range(cores))]

    with tile.TileContext(nc) as tc:
        with ExitStack() as ctx:
            dram = ctx.enter_context(tc.tile_pool(name="dram", bufs=1, space="DRAM"))
            z_bounce = dram.tile([wpc * P, ROWW], F16)  # partition-major rows
            z_full = dram.tile([cores * wpc * P, ROWW], F16)
            lam_loc = dram.tile([wpc * P], F16)

            const = ctx.enter_context(tc.tile_pool(name="const", bufs=1))

            identity = const.tile([P, P], F32)
            make_identity(nc, identity[:])
            iota_i = const.tile([P, P], I32)
            nc.gpsimd.iota(iota_i[:], pattern=[[1, P]], base=0, channel_multiplier=0)
            iota_h = const.tile([P, P], F16)
            nc.vector.tensor_copy(iota_h[:], iota_i[:])

            w_sb = const.tile([P, d_in], F32)
            nc.sync.dma_start(out=w_sb[:], in_=w_d[:, :])
            a_sb = const.tile([P, 2], F32)
            nc.sync.dma_start(out=a_sb[:, 0:1], in_=a_d[0:P, :])
            nc.sync.dma_start(out=a_sb[:, 1:2], in_=a_d[P : 2 * P, :])

            # W_aug_T[kc] = [W.T chunk | v_src | v_dst]  (fp16)
            ctx1 = ctx.enter_context(ExitStack())
            psum1 = ctx1.enter_context(tc.tile_pool(name="psum1", bufs=1, space="PSUM"))
            psumz = ctx1.enter_context(tc.tile_pool(name="psumz", bufs=6, space="PSUM"))
            waug = const.tile([P, kc_n, d_out + 2], F16)
            for kc in range(kc_n):
                ksl = slice(kc * P, (kc + 1) * P)
                pt = psum1.tile([P, P], F32, tag="pt")
                nc.tensor.transpose(pt[:], w_sb[:, ksl], identity[:])
                nc.vector.tensor_copy(waug[:, kc, 0:d_out], pt[:])
                pv = psum1.tile([P, 2], F32, tag="pv")
                nc.tensor.matmul(
                    out=pv[:, 0:1], lhsT=w_sb[:, ksl], rhs=a_sb[:, 0:1],
                    start=True, stop=True,
                )
                nc.tensor.matmul(
                    out=pv[:, 1:2], lhsT=w_sb[:, ksl], rhs=a_sb[:, 1:2],
                    start=True, stop=True,
                )
                nc.vector.tensor_copy(waug[:, kc, d_out : d_out + 2], pv[:])

            # ---- phase 1: z / es / lam accumulate in SBUF, bulk DMAs, AllGather
            h_sb = const.tile([P, kc_n, shard], F16)
            for kc in range(kc_n):
                nc.gpsimd.dma_start(
                    out=h_sb[:, kc, :], in_=h_t[kc * P : (kc + 1) * P, :]
                )

            row_sb = const.tile([P, wpc, ROWW], F16)  # full z_aug rows
            lam_sb = const.tile([P, wpc], F16)
            nc.gpsimd.memset(row_sb[:, :, d_out + 4 : d_out + 5], 1.0)
            nc.gpsimd.memset(row_sb[:, :, d_out + 5 : d_out + 6], 0.0)

            for nt in range(wpc):
                n0 = nt * P
                rows = min(P, shard - n0)
                pz = psumz.tile([P, d_out + 2], F32, tag="pz")
                for kc in range(kc_n):
                    nc.tensor.matmul(
                        out=pz[0:rows, :],
                        lhsT=h_sb[:, kc, n0 : n0 + rows],
                        rhs=waug[:, kc, :],
                        start=(kc == 0),
                        stop=(kc == kc_n - 1),
                    )
                nc.vector.tensor_copy(row_sb[0:rows, nt, 0:d_out], pz[0:rows, 0:d_out])
                nc.scalar.activation(
                    out=row_sb[0:rows, nt, d_out : d_out + 2].bitcast(F32),
                    in_=pz[0:rows, d_out : d_out + 1],
                    func=mybir.ActivationFunctionType.Exp,
                )
                nc.scalar.activation(
                    out=row_sb[0:rows, nt, d_out + 2 : d_out + 4].bitcast(F32),
                    in_=pz[0:rows, d_out : d_out + 1],
                    func=mybir.ActivationFunctionType.Exp, scale=0.01,
                )
                nc.scalar.activation(
                    out=lam_sb[0:rows, nt : nt + 1],
                    in_=pz[0:rows, d_out + 1 : d_out + 2],
                    func=mybir.ActivationFunctionType.Exp, scale=-0.99,
                )

            ctx1.close()

            # bulk row write: partition-major DRAM rows make each partition's
            # whole stream one contiguous run (128 big descriptors)
            zb_view = z_bounce[:, :]  # [wpc*P, ROWW]
            nc.sync.dma_start(
                out=bass.AP(
                    tensor=zb_view.tensor,
                    offset=zb_view.offset,
                    ap=[[ROWW * wpc, P], [ROWW, wpc], [1, ROWW]],
                ),
                in_=row_sb[:, :, :],
            )
            nc.gpsimd.collective_compute(
                "AllGather",
                mybir.AluOpType.bypass,
                replica_groups=rg,
                ins=[z_bounce[:, :]],
                outs=[z_full[:, :]],
            )
            # lam column writes ride the gpsimd queue during the AllGather
            for nt in range(wpc):
                nc.gpsimd.dma_start(
                    out=lam_loc[nt * P : (nt + 1) * P], in_=lam_sb[:, nt]
                )

            # ---- phase 2: per dst-window edge processing
            idxp = ctx.enter_context(tc.tile_pool(name="idxp", bufs=1))
            si_sb = idxp.tile([P, totch], I32)
            sg_sb = idxp.tile([P, totch], F32)
            nc.sync.dma_start(out=si_sb[:], in_=si_d[:, :])
            nc.sync.dma_start(out=sg_sb[:], in_=sg_d[:, :])

            gp = ctx.enter_context(tc.tile_pool(name="gp", bufs=24))
            sp = ctx.enter_context(tc.tile_pool(name="sp", bufs=3))
            maskp = ctx.enter_context(tc.tile_pool(name="maskp", bufs=3))
            selp = ctx.enter_context(tc.tile_pool(name="selp", bufs=8))
            op = ctx.enter_context(tc.tile_pool(name="op", bufs=3))
            psum2 = ctx.enter_context(tc.tile_pool(name="psum2", bufs=4, space="PSUM"))

            capmax = int(max(cap_w))

            for w in range(wpc):
                n0 = w * P
                rows = min(P, shard - n0)
                capw = int(cap_w[w])
                st = int(starts[w])

                gs = []
                for c in range(capw):
                    g = gp.tile([P, ROWW], F16, tag="g")
                    nc.gpsimd.indirect_dma_start(
                        out=g[:, :],
                        out_offset=None,
                        in_=z_full[:, :],
                        in_offset=bass.IndirectOffsetOnAxis(
                            ap=si_sb[:, st + c : st + c + 1], axis=0
                        ),
                    )
                    gs.append(g)

                # lam row broadcast into all partitions: LAM[p, d] = lam[n0+d]
                lamb = sp.tile([P, P], F16, tag="lamb")
                lam_bcast = bass.AP(
                    tensor=lam_loc.tensor,
                    offset=lam_loc.offset + n0,
                    ap=[[0, P], [1, rows]],
                )
                nc.sync.dma_start(out=lamb[:, 0:rows], in_=lam_bcast)

                # mask[p, c, d] = (sg[p, st+c] == d)
                mask = maskp.tile([P, capmax, P], F16, tag="mask")
                nc.vector.tensor_tensor(
                    out=mask[:, 0:capw, :],
                    in0=iota_h[:, None, :].broadcast_to([P, capw, P]),
                    in1=sg_sb[:, st : st + capw, None].broadcast_to([P, capw, P]),
                    op=mybir.AluOpType.is_equal,
                )

                po = psum2.tile([P, ROWW], F32, tag="po")
                for c in range(capw):
                    g = gs[c]
                    m = selp.tile([P, P], F16, tag="m")
                    nc.vector.tensor_scalar(
                        out=m[:],
                        in0=lamb[:],
                        scalar1=g[:, d_out + 2 : d_out + 4].bitcast(F32),
                        scalar2=g[:, d_out : d_out + 2].bitcast(F32),
                        op0=mybir.AluOpType.mult,
                        op1=mybir.AluOpType.max,
                    )
                    sel = selp.tile([P, P], F16, tag="sel")
                    nc.vector.tensor_tensor(
                        out=sel[:], in0=mask[:, c, :], in1=m[:],
                        op=mybir.AluOpType.mult,
                    )
                    nc.tensor.matmul(
                        out=po[:],
                        lhsT=sel[:],
                        rhs=g[:, :],
                        start=(c == 0),
                        stop=(c == capw - 1),
                    )

                den = op.tile([P, 1], F32, tag="den")
                nc.scalar.activation(
                    out=den[:], in_=po[:, d_out + 4 : d_out + 5],
                    func=mybir.ActivationFunctionType.Copy, bias=1e-6,
                )
                rec = op.tile([P, 1], F32, tag="rec")
                nc.vector.reciprocal(rec[:], den[:])
                ot = op.tile([P, d_out], F32, tag="ot")
                nc.scalar.activation(
                    out=ot[:], in_=po[:, 0:d_out],
                    func=mybir.ActivationFunctionType.Copy, scale=rec[:, 0:1],
                )
                nc.sync.dma_start(out=out_d[n0 : n0 + rows, :], in_=ot[0:rows, :])

    nc.compile()
    return nc


# ---------------------------------------------------------------- driver
def prepare(h, W, a, src, dst):
    h = np.asarray(h, dtype=np.float32)
    W = np.asarray(W, dtype=np.float32)
    a = np.asarray(a, dtype=np.float32)
    n_nodes = h.shape[0]
    shard = n_nodes // CORES

    cap_w, arrs = preprocess_indices(src, dst, n_nodes=n_nodes)
    key = (cap_w, n_nodes, h.shape[1], W.shape[0])
    if key not in _PROGRAM_CACHE:
        _PROGRAM_CACHE[key] = build_program(
            cap_w, n_nodes=n_nodes, d_in=h.shape[1], d_out=W.shape[0]
        )
    nc = _PROGRAM_CACHE[key]

    in_maps = []
    for c in range(CORES):
        h_t_c = np.ascontiguousarray(h[c * shard : (c + 1) * shard].T)
        m = {"h_t": h_t_c, "W": W, "a": a}
        m.update(arrs[c])
        in_maps.append(m)
    return nc, in_maps


def kernel(h, W, a, src, dst):
    from concourse.bass_utils import run_bass_kernel_spmd

    nc, in_maps = prepare(h, W, a, src, dst)
    res = run_bass_kernel_spmd(nc, in_maps, core_ids=list(range(CORES)))
    outs = [res.results[c]["out"] for c in range(CORES)]
    return np.ascontiguousarray(np.concatenate(outs, axis=0).astype(np.float32))
